# revision 1
# baseline (speedup 1.0000x reference)
"""Trainium2 Bass kernel for block-adapter Linear (nn_Linear_20847771255232).

Math:
    y = x @ W_base^T + b_base + s * adapter(x)
where the block-structured adapter folds into an effective weight:
    W_eff[o_blk*64+e, i*64+d] = W_base[...] + s * U[d, i, o_blk] * V[o_blk, d, e]
(no reduction in the adapter -- pure elementwise construction), so the whole
problem is ONE 4096x4096x4096 GEMM with an on-device-constructed bf16 weight.

Sharding (8 cores): 4-way data parallel over tokens (m) x 2-way tensor
parallel over out features (o).

Per-core pipeline:
  - x rows: SWDGE cast-DMA (f32->bf16), xbar transpose (ACT HWDGE queue) into
    resident xT [128k, ki, m].
  - W rows streamed per 256-wide o chunk: SWDGE cast-DMA, xbar transpose
    (SP HWDGE queue) into wts [128k, ki, o5]; adapter added in place
    (U broadcast over e via a small PE matmul against a block-selector
    "E ones" matrix, times V replicated over both 64-partition halves; DVE).
  - GEMM: mt-pairs accumulate (bias seeded via K=1 matmul) in shared PSUM
    banks; the next chunk's adapter work is interleaved into the GEMM
    emission so PE/DVE pipelines overlap across chunks; DVE drains, SWDGE
    stores. HWDGE queues carry ONLY transposes (mixing copy DMAs onto an
    xbar queue corrupts transfers).
"""

import numpy as np

BLOCK = 64
M_TOT, K_TOT, O_TOT = 4096, 4096, 4096
GRID_M, GRID_O = 4, 2
M_C, O_C = M_TOT // GRID_M, O_TOT // GRID_O  # 1024, 2048
O5 = 256

_CACHE = {}


def build_program(m_c=M_C, k=K_TOT, o_c=O_C, o5=O5, num_devices=8, debug=False):
    import concourse.bacc as bacc
    import concourse.bass as bass
    import concourse.mybir as mybir
    import concourse.tile as tile

    f32 = mybir.dt.float32
    bf16 = mybir.dt.bfloat16

    S = k // 128          # k-stripes of 128
    NB_I = k // BLOCK     # input blocks
    NB_O = o_c // BLOCK   # output blocks on this core (mult of 32)
    MT = m_c // 128
    NOC = o_c // o5
    WPC = o5 // 128
    assert NB_O % 32 == 0 and NB_O <= 128

    nc = bacc.Bacc(
        "TRN2",
        target_bir_lowering=False,
        debug=debug,
        num_devices=num_devices,
    )
    x_d = nc.dram_tensor("x_s", [m_c, k], f32, kind="ExternalInput").ap()
    w_d = nc.dram_tensor("w_s", [o_c, k], f32, kind="ExternalInput").ap()
    b_d = nc.dram_tensor("b_s", [o_c], f32, kind="ExternalInput").ap()
    u_d = nc.dram_tensor("u_s", [BLOCK, NB_I, NB_O], f32, kind="ExternalInput").ap()
    v_d = nc.dram_tensor("v_s", [NB_O, BLOCK, BLOCK], f32, kind="ExternalInput").ap()
    s_d = nc.dram_tensor("s_s", [1], f32, kind="ExternalInput").ap()
    y_d = nc.dram_tensor("y_s", [m_c, o_c], f32, kind="ExternalOutput").ap()

    with tile.TileContext(nc) as tc:
        with (
            tc.tile_pool(name="const", bufs=1) as cpool,
            tc.tile_pool(name="xstage", bufs=3) as xstpool,
            tc.tile_pool(name="wstage", bufs=3) as wstpool,
            tc.tile_pool(name="xt", bufs=1) as xtpool,
            tc.tile_pool(name="wt", bufs=3) as wtpool,
            tc.tile_pool(name="adap", bufs=3) as apool,
            tc.tile_pool(name="outp", bufs=3) as opool,
            tc.tile_pool(name="psum_mm", bufs=4, space=bass.MemorySpace.PSUM) as pspool,
            tc.tile_pool(name="psum_ub", bufs=4, space=bass.MemorySpace.PSUM) as ubpool,
        ):
            # ------- W chunk loads (SWDGE cast + SP-queue xbar) -------
            def load_w_chunk(oc):
                wts = wtpool.tile([128, S, o5], bf16, tag="wt")
                for wc in range(WPC):
                    wbf = wstpool.tile([128, k], bf16, tag="wstage")
                    nc.gpsimd.dma_start(
                        wbf[:],
                        w_d[oc * o5 + wc * 128 : oc * o5 + (wc + 1) * 128, :],
                    )
                    nc.sync.dma_start_transpose(
                        wts[:, :, wc * 128 : (wc + 1) * 128], wbf[:]
                    )
                return wts

            # ------- x tile loads (SWDGE cast + ACT-queue xbar) -------
            xT = xtpool.tile([128, S, m_c], bf16)

            def load_x_tile(mt):
                xbf = xstpool.tile([128, k], bf16, tag="xstage")
                nc.gpsimd.dma_start(xbf[:], x_d[mt * 128 : (mt + 1) * 128, :])
                nc.scalar.dma_start_transpose(
                    xT[:, :, mt * 128 : (mt + 1) * 128], xbf[:]
                )

            # ---- U path first: it heads the startup critical chain ----
            ones_f32 = cpool.tile([1, 128], f32)
            nc.vector.memset(ones_f32[:], 1.0)
            ones_bf = cpool.tile([1, 128], bf16)
            nc.vector.tensor_copy(ones_bf[:], ones_f32[:])

            s_sb = cpool.tile([1, 1], f32)
            nc.gpsimd.dma_start(s_sb[:], s_d[None, :])
            u_bf = cpool.tile([BLOCK, NB_I * NB_O], bf16)
            nc.gpsimd.dma_start(u_bf[:], u_d.rearrange("d i o -> d (i o)"))
            # V: one strided gather into the lower half; replicate on GpSimd
            # (keeps the 128B-packet gather off the queue's hot window)
            v_rep = cpool.tile([128, o_c], bf16)
            nc.gpsimd.dma_start(
                v_rep[0:BLOCK, :].rearrange("d (j e) -> d j e", e=BLOCK),
                v_d.rearrange("j d e -> d j e"),
            )

            # scale U by s = S[0] (SCALING == 1.0)
            s_ps = ubpool.tile([BLOCK, 1], f32, tag="ub")
            nc.tensor.matmul(
                s_ps[:], ones_f32[:, 0:BLOCK], s_sb[:], start=True, stop=True
            )
            s_col = cpool.tile([BLOCK, 1], f32)
            nc.vector.tensor_copy(s_col[:], s_ps[:])
            nc.vector.tensor_scalar_mul(u_bf[:], u_bf[:], s_col[:])

            # start the big loads
            wts_q = [load_w_chunk(0)]
            for mt in range(min(2, MT)):
                load_x_tile(mt)

            # ---------------- remaining constants ----------------
            b_bf = cpool.tile([1, o_c], bf16)
            nc.gpsimd.dma_start(b_bf[:], b_d[None, :])  # cast f32->bf16
            nc.gpsimd.tensor_copy(v_rep[BLOCK : 2 * BLOCK, :], v_rep[0:BLOCK, :])

            # U_colsT[j, ki*128 + h*64 + d] = s * U[d, 2ki+h, j]   (bf16)
            ucolsT = cpool.tile([NB_O, S * 128], bf16)
            for ki in range(S):
                for h in range(2):
                    i = 2 * ki + h
                    for r in range(2):  # 32-row halves of d
                        for c in range(NB_O // 32):
                            nc.vector.transpose(
                                ucolsT[
                                    32 * c : 32 * c + 32,
                                    ki * 128 + h * 64 + 32 * r : ki * 128
                                    + h * 64
                                    + 32 * r
                                    + 32,
                                ],
                                u_bf[
                                    32 * r : 32 * r + 32,
                                    i * NB_O + 32 * c : i * NB_O + 32 * c + 32,
                                ],
                            )

            # E-ones: row j has ones on cols [j*64, (j+1)*64).
            eones = cpool.tile([NB_O, o_c], bf16)
            nc.gpsimd.memset(eones[:], 1.0)
            eones3 = eones[:].rearrange("j (jc e) -> j jc e", e=BLOCK)
            nc.gpsimd.affine_select(
                out=eones3,
                in_=eones3,
                compare_op=mybir.AluOpType.is_ge,
                fill=0.0,
                base=0,
                pattern=[[-1, NB_O], [0, BLOCK]],
                channel_multiplier=1,
            )
            nc.gpsimd.affine_select(
                out=eones3,
                in_=eones3,
                compare_op=mybir.AluOpType.is_ge,
                fill=0.0,
                base=0,
                pattern=[[1, NB_O], [0, BLOCK]],
                channel_multiplier=-1,
            )

            # rest of the big loads
            wts_q.append(load_w_chunk(1) if NOC > 1 else None)
            for mt in range(2, MT):
                load_x_tile(mt)

            # ------- adapter: wts[:, ki, :] += s*U[d,i,j]*V[j,d,e] -------
            # one ki-pair chunk: bcast-matmuls + [128, 512] DVE mul/add
            def adapter_pair(oc, wts, kp):
                vb = (
                    v_rep[:, oc * o5 : (oc + 1) * o5]
                    .rearrange("p (one f) -> p one f", one=1)
                    .broadcast_to([128, 2, o5])
                )
                ub2 = ubpool.tile([128, 2, o5], f32, tag="ub")
                for h2 in range(2):
                    ki = 2 * kp + h2
                    nc.tensor.matmul(
                        ub2[:, h2, :],
                        ucolsT[:, ki * 128 : (ki + 1) * 128],
                        eones[:, oc * o5 : (oc + 1) * o5],
                        start=True,
                        stop=True,
                    )
                ad2 = apool.tile([128, 2, o5], bf16, tag="adap")
                nc.vector.tensor_mul(ad2[:], ub2[:], vb)
                nc.vector.tensor_add(
                    wts[:, 2 * kp : 2 * kp + 2, :],
                    ad2[:],
                    wts[:, 2 * kp : 2 * kp + 2, :],
                )

            for kp in range(S // 2):
                adapter_pair(0, wts_q[0], kp)

            # ------- main loop: GEMM(oc) with adapter(oc+1) interleaved ----
            mt_groups = [
                list(range(g, min(g + 2, MT))) for g in range(0, MT, 2)
            ]
            NPAIR = len(mt_groups)
            KPP = (S // 2 + NPAIR - 1) // NPAIR  # adapter pairs per mt-pair
            for oc in range(NOC):
                wts_cur = wts_q[0]
                wts_q = [wts_q[1], load_w_chunk(oc + 2) if oc + 2 < NOC else None]
                wts_next = wts_q[0] if oc + 1 < NOC else None

                for half, group in enumerate(mt_groups):
                    ps2 = pspool.tile([128, len(group), o5], f32, tag="ps")
                    for j, mt in enumerate(group):
                        nc.tensor.matmul(
                            ps2[:, j, :],
                            ones_bf[:],
                            b_bf[:, oc * o5 : (oc + 1) * o5],
                            start=True,
                            stop=False,
                        )
                        for ki in range(S):
                            nc.tensor.matmul(
                                ps2[:, j, :],
                                xT[:, ki, mt * 128 : (mt + 1) * 128],
                                wts_cur[:, ki, :],
                                start=False,
                                stop=(ki == S - 1),
                            )
                    # interleave next chunk's adapter work
                    if wts_next is not None:
                        for kp in range(KPP * half, min(KPP * (half + 1), S // 2)):
                            adapter_pair(oc + 1, wts_next, kp)
                    # drain the mt-pair (+stores via SWDGE)
                    osb = opool.tile([128, len(group), o5], f32, tag="o")
                    nc.vector.tensor_copy(osb[:], ps2[:])
                    nc.gpsimd.dma_start(
                        y_d[
                            group[0] * 128 : (group[-1] + 1) * 128,
                            oc * o5 : (oc + 1) * o5,
                        ].rearrange("(j p) c -> p j c", p=128),
                        osb[:],
                    )

    nc.compile()
    return nc


def _get_program():
    key = "full"
    if key not in _CACHE:
        _CACHE[key] = build_program()
    return _CACHE[key]


def kernel(x, W_base, b_base, U, V, S):
    from concourse import bass_utils

    x = np.asarray(x, dtype=np.float32)
    W_base = np.asarray(W_base, dtype=np.float32)
    b_base = np.asarray(b_base, dtype=np.float32)
    U = np.asarray(U, dtype=np.float32)
    V = np.asarray(V, dtype=np.float32)
    S = np.asarray(S, dtype=np.float32)

    B, N, DIN = x.shape
    xf = np.ascontiguousarray(x.reshape(B * N, DIN))

    nc = _get_program()

    in_maps = []
    for c in range(8):
        mc, oc = divmod(c, GRID_O)
        nbo = O_C // BLOCK
        in_maps.append(
            {
                "x_s": np.ascontiguousarray(xf[mc * M_C : (mc + 1) * M_C]),
                "w_s": np.ascontiguousarray(W_base[oc * O_C : (oc + 1) * O_C]),
                "b_s": np.ascontiguousarray(b_base[oc * O_C : (oc + 1) * O_C]),
                "u_s": np.ascontiguousarray(U[:, :, oc * nbo : (oc + 1) * nbo]),
                "v_s": np.ascontiguousarray(V[oc * nbo : (oc + 1) * nbo]),
                "s_s": S,
            }
        )

    res = bass_utils.run_bass_kernel_spmd(nc, in_maps, core_ids=list(range(8)))

    y = np.empty((B * N, O_TOT), dtype=np.float32)
    for c in range(8):
        mc, oc = divmod(c, GRID_O)
        y[mc * M_C : (mc + 1) * M_C, oc * O_C : (oc + 1) * O_C] = res.results[c]["y_s"]
    return y.reshape(B, N, O_TOT)



# revision 4
# speedup vs baseline: 1.5004x; 1.5004x over previous
"""Trainium2 Bass kernel for block-adapter Linear (nn_Linear_20847771255232).

Math:
    y = x @ W_base^T + b_base + s * adapter(x)
where the block-structured adapter folds into an effective weight:
    W_eff[j*64+e, i*64+d] = W_base[...] + s * U[d, i, j] * V[j, d, e]
(pure elementwise construction), so the whole problem is ONE
4096x4096x4096 GEMM with an on-device-constructed bf16 weight.

Sharding (8 cores): 4-way data parallel over tokens (m) x 2-way tensor
parallel over out features (o).

v2 design (vs v1 baseline at ~507us):
  - All inputs are re-laid-out HOST-side to k-major (pure relayout, no
    arithmetic): x^T [K, M_C], W^T pre-chunked [NOC, 128p, S*O5], U/V in
    the exact SBUF layouts the adapter needs. This removes ALL on-device
    transposes (v1 spent ~117us of DMA-queue time on xbar transposes and
    serialized stage->transpose->adapter chains).
  - SWDGE cast-DMAs (f32->bf16) land data directly in matmul layout with
    fully contiguous per-partition descriptors.
  - Adapter is DVE-only: one broadcast-AP tensor_mul (U bcast over e, V
    bcast over ki) + one tensor_add per ki-chunk. No PE matmuls (v1 spent
    38us of PE on broadcast matmuls).
  - GEMM uses N=512 matmuls (one PSUM bank per (mt, oc4)); bias is fused
    into the PSUM->SBUF drain via scalar_tensor_tensor against a
    PE-replicated bias tile, so no K=1 seeding matmuls in the hot loop.
  - Phase 1 (first 512 output cols) runs ki-outer across all 8 m-tiles
    (8 PSUM banks) so the PE chases the streaming x quads; phases 2-4 run
    mt-outer with W chunks double-buffered and the next chunk's adapter
    applied on DVE during the previous sweep.
"""

import numpy as np

BLOCK = 64
M_TOT, K_TOT, O_TOT = 4096, 4096, 4096
GRID_M, GRID_O = 4, 2
M_C, O_C = M_TOT // GRID_M, O_TOT // GRID_O  # 1024, 2048
S = K_TOT // 128          # 32 k-stripes
MT = M_C // 128           # 8 m-tiles
O5 = 512                  # matmul free dim = one PSUM bank of f32
NOC = O_C // O5           # 4 weight chunks
NBO = O_C // BLOCK        # 32 output blocks per core

_CACHE = {}


def build_program(num_devices=8, debug=False):
    import concourse.bacc as bacc
    import concourse.bass as bass
    import concourse.mybir as mybir
    import concourse.tile as tile

    f32 = mybir.dt.float32
    bf16 = mybir.dt.bfloat16
    mult = mybir.AluOpType.mult
    addop = mybir.AluOpType.add

    nc = bacc.Bacc(
        "TRN2",
        target_bir_lowering=False,
        debug=debug,
        num_devices=num_devices,
    )
    xt_d = nc.dram_tensor("xt", [K_TOT, M_C], f32, kind="ExternalInput").ap()
    w_d = nc.dram_tensor("wc", [NOC, 128, S * O5], f32, kind="ExternalInput").ap()
    b_d = nc.dram_tensor("bb", [O_C], f32, kind="ExternalInput").ap()
    u_d = nc.dram_tensor("uc", [128, S * NBO], f32, kind="ExternalInput").ap()
    v_d = nc.dram_tensor("vr", [128, O_C], f32, kind="ExternalInput").ap()
    s_d = nc.dram_tensor("ss", [1], f32, kind="ExternalInput").ap()
    y_d = nc.dram_tensor("yy", [M_C, O_C], f32, kind="ExternalOutput").ap()

    with tile.TileContext(nc) as tc:
        with (
            tc.tile_pool(name="const", bufs=1) as cpool,
            tc.tile_pool(name="xt", bufs=1) as xtpool,
            tc.tile_pool(name="wt", bufs=2) as wtpool,
            tc.tile_pool(name="adap", bufs=2) as apool,
            tc.tile_pool(name="outp", bufs=4) as opool,
            tc.tile_pool(name="ps", bufs=8, space=bass.MemorySpace.PSUM) as pspool,
        ):
            # ---------------- tiny constants ----------------
            ucols = cpool.tile([128, S * NBO], bf16)
            nc.gpsimd.dma_start(ucols[:], u_d)
            v_bf = cpool.tile([128, O_C], bf16)
            nc.gpsimd.dma_start(v_bf[:], v_d)
            s_sb = cpool.tile([1, 1], f32)
            nc.gpsimd.dma_start(s_sb[:], s_d[None, :])
            b_sb = cpool.tile([1, O_C], bf16)
            nc.gpsimd.dma_start(b_sb[:], b_d[None, :])  # cast f32->bf16

            # broadcast s = S[0] to a [128,1] column via K=1 matmul
            ones_f32 = cpool.tile([1, 128], f32)
            nc.vector.memset(ones_f32[:], 1.0)
            s_ps = pspool.tile([128, O5], f32, tag="ps")
            nc.tensor.matmul(s_ps[:, 0:1], ones_f32[:], s_sb[:], start=True, stop=True)
            s_col = cpool.tile([128, 1], f32)
            nc.vector.tensor_copy(s_col[:], s_ps[:, 0:1])
            # scale U columns by s in place (SCALING == 1.0)
            nc.vector.tensor_scalar_mul(ucols[:], ucols[:], s_col[:])

            # ---------------- big loads ----------------
            # W chunk 0 first (heads the adapter+GEMM critical chain)
            wts = []
            w0 = wtpool.tile([128, S, O5], bf16, tag="wt")
            nc.gpsimd.dma_start(w0[:].rearrange("p k o -> p (k o)"), w_d[0])
            wts.append(w0)

            # x quads (4 stripes each); W1 halves slotted late so they
            # don't starve the phase-1 x stream
            xT = xtpool.tile([128, S, M_C], bf16)
            w1 = wtpool.tile([128, S, O5], bf16, tag="wt")
            for q in range(8):
                nc.gpsimd.dma_start(
                    xT[:, 4 * q : 4 * q + 4, :],
                    xt_d[q * 512 : (q + 1) * 512, :].rearrange(
                        "(kk p) m -> p kk m", p=128
                    ),
                )
                if q == 5:
                    nc.gpsimd.dma_start(
                        w1[:, 0:16, :].rearrange("p k o -> p (k o)"),
                        w_d[1][:, 0 : 16 * O5],
                    )
                if q == 7:
                    nc.gpsimd.dma_start(
                        w1[:, 16:32, :].rearrange("p k o -> p (k o)"),
                        w_d[1][:, 16 * O5 :],
                    )
            wts.append(w1)

            # ---------------- bias replicated to 128 partitions ----------------
            ones_bf = cpool.tile([1, 128], bf16)
            nc.vector.tensor_copy(ones_bf[:], ones_f32[:])
            b_rep = cpool.tile([128, O_C], bf16)
            for c4 in range(NOC):
                bp = pspool.tile([128, O5], f32, tag="ps")
                nc.tensor.matmul(
                    bp[:],
                    ones_bf[:],
                    b_sb[:, c4 * O5 : (c4 + 1) * O5],
                    start=True,
                    stop=True,
                )
                nc.vector.tensor_copy(b_rep[:, c4 * O5 : (c4 + 1) * O5], bp[:])

            # ---------------- adapter: wt[p, ki, j*64+e] += s*U*V ----------------
            u3 = ucols[:].rearrange("p (k j) -> p k j", j=NBO)

            def adapter(c, wt, q):  # ki chunk [8q, 8q+8)
                ad = apool.tile([128, 8, O5], bf16, tag="ad")
                ub = (
                    u3[:, q * 8 : (q + 1) * 8, c * 8 : (c + 1) * 8]
                    .rearrange("p k (j one) -> p k j one", one=1)
                    .broadcast_to([128, 8, 8, BLOCK])
                )
                vb = (
                    v_bf[:, c * O5 : (c + 1) * O5]
                    .rearrange("p (one j e) -> p one j e", one=1, j=8)
                    .broadcast_to([128, 8, 8, BLOCK])
                )
                nc.vector.tensor_mul(
                    ad[:].rearrange("p k (j e) -> p k j e", e=BLOCK), ub, vb
                )
                nc.vector.tensor_add(
                    wt[:, q * 8 : (q + 1) * 8, :], ad[:], wt[:, q * 8 : (q + 1) * 8, :]
                )

            for q in range(4):
                adapter(0, w0, q)

            # ---------------- drain helper (fused bias add) ----------------
            def drain(ps, mt, c4):
                osb = opool.tile([128, O5], f32, tag="o")
                nc.vector.scalar_tensor_tensor(
                    osb[:],
                    ps[:],
                    1.0,
                    b_rep[:, c4 * O5 : (c4 + 1) * O5],
                    op0=mult,
                    op1=addop,
                )
                nc.sync.dma_start(
                    y_d[mt * 128 : (mt + 1) * 128, c4 * O5 : (c4 + 1) * O5], osb[:]
                )

            # ---------------- phase 1: oc4=0, ki-outer (chases x quads) --------
            ps1 = [pspool.tile([128, O5], f32, tag="ps", name=f"ps1_{i}") for i in range(MT)]
            for ki in range(S):
                for mt in range(MT):
                    nc.tensor.matmul(
                        ps1[mt][:],
                        xT[:, ki, mt * 128 : (mt + 1) * 128],
                        w0[:, ki, :],
                        start=(ki == 0),
                        stop=(ki == S - 1),
                    )
            # adapter for W1 runs on DVE while phase-1 GEMM streams
            for q in range(4):
                adapter(1, w1, q)
            for mt in range(MT):
                drain(ps1[mt], mt, 0)

            # ---------------- phases 2-4: mt-outer, W double-buffered ----------
            for c4 in range(1, NOC):
                for mt in range(MT):
                    ps = pspool.tile([128, O5], f32, tag="ps")
                    for ki in range(S):
                        nc.tensor.matmul(
                            ps[:],
                            xT[:, ki, mt * 128 : (mt + 1) * 128],
                            wts[c4][:, ki, :],
                            start=(ki == 0),
                            stop=(ki == S - 1),
                        )
                    drain(ps, mt, c4)
                    if mt == 1 and c4 + 1 < NOC:
                        wn = wtpool.tile([128, S, O5], bf16, tag="wt")
                        nc.gpsimd.dma_start(
                            wn[:].rearrange("p k o -> p (k o)"), w_d[c4 + 1]
                        )
                        for q in range(4):
                            adapter(c4 + 1, wn, q)
                        wts.append(wn)

    nc.compile()
    return nc


def _get_program():
    key = "full"
    if key not in _CACHE:
        _CACHE[key] = build_program()
    return _CACHE[key]


def _prep_in_maps(x, W_base, b_base, U, V, S):
    """Host-side sharding + pure relayout (no arithmetic on values)."""
    B, N, DIN = x.shape
    xf = np.ascontiguousarray(x.reshape(B * N, DIN))

    ns = K_TOT // 128  # stripe count (module-level S is shadowed by scale input)
    # per-oc tensors (shared by the 4 data-parallel cores in each column)
    w_oc, b_oc, u_oc, v_oc = [], [], [], []
    for oc in range(GRID_O):
        WT = W_base[oc * O_C : (oc + 1) * O_C, :].T  # [K, O_C] view
        wc = np.ascontiguousarray(
            WT.reshape(ns, 128, NOC, O5).transpose(2, 1, 0, 3)
        ).reshape(NOC, 128, ns * O5)
        w_oc.append(wc)
        b_oc.append(np.ascontiguousarray(b_base[oc * O_C : (oc + 1) * O_C]))
        Uj = U[:, :, oc * NBO : (oc + 1) * NBO]  # [64 d, 64 i, NBO j]
        uc = np.ascontiguousarray(
            Uj.transpose(1, 0, 2).reshape(ns, 2, BLOCK, NBO).transpose(1, 2, 0, 3)
        ).reshape(128, ns * NBO)
        u_oc.append(uc)
        Vj = V[oc * NBO : (oc + 1) * NBO]  # [NBO j, 64 d, 64 e]
        vt = Vj.transpose(1, 0, 2).reshape(BLOCK, O_C)  # [d, j*64+e]
        v_oc.append(np.ascontiguousarray(np.concatenate([vt, vt], axis=0)))

    xt_mc = [
        np.ascontiguousarray(xf[mc * M_C : (mc + 1) * M_C, :].T)
        for mc in range(GRID_M)
    ]

    in_maps = []
    for c in range(8):
        mc, oc = divmod(c, GRID_O)
        in_maps.append(
            {
                "xt": xt_mc[mc],
                "wc": w_oc[oc],
                "bb": b_oc[oc],
                "uc": u_oc[oc],
                "vr": v_oc[oc],
                "ss": np.ascontiguousarray(S),
            }
        )
    return in_maps


def kernel(x, W_base, b_base, U, V, S):
    from concourse import bass_utils

    x = np.asarray(x, dtype=np.float32)
    W_base = np.asarray(W_base, dtype=np.float32)
    b_base = np.asarray(b_base, dtype=np.float32)
    U = np.asarray(U, dtype=np.float32)
    V = np.asarray(V, dtype=np.float32)
    S = np.asarray(S, dtype=np.float32)

    B, N, DIN = x.shape
    nc = _get_program()
    in_maps = _prep_in_maps(x, W_base, b_base, U, V, S)
    res = bass_utils.run_bass_kernel_spmd(nc, in_maps, core_ids=list(range(8)))

    y = np.empty((M_TOT, O_TOT), dtype=np.float32)
    for c in range(8):
        mc, oc = divmod(c, GRID_O)
        y[mc * M_C : (mc + 1) * M_C, oc * O_C : (oc + 1) * O_C] = res.results[c]["yy"]
    return y.reshape(B, N, O_TOT)


# revision 5
# speedup vs baseline: 1.6849x; 1.1230x over previous
"""Trainium2 Bass kernel for block-adapter Linear (nn_Linear_20847771255232).

Math:
    y = x @ W_base^T + b_base + s * adapter(x)
where the block-structured adapter folds into an effective weight:
    W_eff[j*64+e, i*64+d] = W_base[...] + s * U[d, i, j] * V[j, d, e]
(pure elementwise construction), so the whole problem is ONE
4096x4096x4096 GEMM with an on-device-constructed bf16 weight.

Sharding (8 cores): 4-way data parallel over tokens (m) x 2-way tensor
parallel over out features (o).

v2 design (vs v1 baseline at ~507us):
  - All inputs are re-laid-out HOST-side to k-major (pure relayout, no
    arithmetic): x^T [K, M_C], W^T pre-chunked [NOC, 128p, S*O5], U/V in
    the exact SBUF layouts the adapter needs. This removes ALL on-device
    transposes (v1 spent ~117us of DMA-queue time on xbar transposes and
    serialized stage->transpose->adapter chains).
  - SWDGE cast-DMAs (f32->bf16) land data directly in matmul layout with
    fully contiguous per-partition descriptors.
  - Adapter is DVE-only: one broadcast-AP tensor_mul (U bcast over e, V
    bcast over ki) + one tensor_add per ki-chunk. No PE matmuls (v1 spent
    38us of PE on broadcast matmuls).
  - GEMM uses N=512 matmuls (one PSUM bank per (mt, oc4)); bias is fused
    into the PSUM->SBUF drain via scalar_tensor_tensor against a
    PE-replicated bias tile, so no K=1 seeding matmuls in the hot loop.
  - Phase 1 (first 512 output cols) runs ki-outer across all 8 m-tiles
    (8 PSUM banks) so the PE chases the streaming x quads; phases 2-4 run
    mt-outer with W chunks double-buffered and the next chunk's adapter
    applied on DVE during the previous sweep.
"""

import numpy as np

BLOCK = 64
M_TOT, K_TOT, O_TOT = 4096, 4096, 4096
GRID_M, GRID_O = 4, 2
M_C, O_C = M_TOT // GRID_M, O_TOT // GRID_O  # 1024, 2048
S = K_TOT // 128          # 32 k-stripes
MT = M_C // 128           # 8 m-tiles
O5 = 512                  # matmul free dim = one PSUM bank of f32
NOC = O_C // O5           # 4 weight chunks
NBO = O_C // BLOCK        # 32 output blocks per core

_CACHE = {}


def build_program(num_devices=8, debug=False):
    import concourse.bacc as bacc
    import concourse.bass as bass
    import concourse.mybir as mybir
    import concourse.tile as tile

    f32 = mybir.dt.float32
    bf16 = mybir.dt.bfloat16
    mult = mybir.AluOpType.mult
    addop = mybir.AluOpType.add

    nc = bacc.Bacc(
        "TRN2",
        target_bir_lowering=False,
        debug=debug,
        num_devices=num_devices,
    )
    xt_d = nc.dram_tensor("xt", [K_TOT, M_C], f32, kind="ExternalInput").ap()
    w_d = nc.dram_tensor("wc", [NOC, 128, S * O5], f32, kind="ExternalInput").ap()
    b_d = nc.dram_tensor("bb", [O_C], f32, kind="ExternalInput").ap()
    u_d = nc.dram_tensor("uc", [128, S * NBO], f32, kind="ExternalInput").ap()
    v_d = nc.dram_tensor("vr", [128, O_C], f32, kind="ExternalInput").ap()
    s_d = nc.dram_tensor("ss", [1], f32, kind="ExternalInput").ap()
    y_d = nc.dram_tensor("yy", [M_C, O_C], f32, kind="ExternalOutput").ap()

    with tile.TileContext(nc) as tc:
        with (
            tc.tile_pool(name="const", bufs=1) as cpool,
            tc.tile_pool(name="xt", bufs=1) as xtpool,
            tc.tile_pool(name="wt", bufs=2) as wtpool,
            tc.tile_pool(name="adap", bufs=2) as apool,
            tc.tile_pool(name="outp", bufs=4) as opool,
            tc.tile_pool(name="ps", bufs=8, space=bass.MemorySpace.PSUM) as pspool,
        ):
            # ---------------- tiny constants ----------------
            ucols = cpool.tile([128, S * NBO], bf16)
            nc.gpsimd.dma_start(ucols[:], u_d)
            v_bf = cpool.tile([128, O_C], bf16)
            nc.gpsimd.dma_start(v_bf[:], v_d)
            s_sb = cpool.tile([1, 1], f32)
            nc.gpsimd.dma_start(s_sb[:], s_d[None, :])
            b_sb = cpool.tile([1, O_C], bf16)
            nc.gpsimd.dma_start(b_sb[:], b_d[None, :])  # cast f32->bf16

            # broadcast s = S[0] to a [128,1] column via K=1 matmul
            ones_f32 = cpool.tile([1, 128], f32)
            nc.vector.memset(ones_f32[:], 1.0)
            s_ps = pspool.tile([128, O5], f32, tag="ps")
            nc.tensor.matmul(s_ps[:, 0:1], ones_f32[:], s_sb[:], start=True, stop=True)
            s_col = cpool.tile([128, 1], f32)
            nc.vector.tensor_copy(s_col[:], s_ps[:, 0:1])
            # scale U columns by s in place (SCALING == 1.0)
            nc.vector.tensor_scalar_mul(ucols[:], ucols[:], s_col[:])

            # ---------------- big loads (chained, 2-deep) ----------------
            # The SDMA engines round-robin across ALL queued transfers, so
            # naively queueing everything makes the first-needed tensors
            # finish last-ish. Chain the loads with tiny gpsimd "guard"
            # reads: each guard blocks the (in-order) gpsimd queue until a
            # prior transfer lands, keeping ~2 DMAs in flight and the
            # completion order equal to the consumption order.
            gscratch = cpool.tile([1, 1], bf16)

            def guard(ap_slice):
                nc.gpsimd.tensor_copy(gscratch[:], ap_slice)

            wts = []
            w0 = wtpool.tile([128, S, O5], bf16, tag="wt")
            nc.gpsimd.dma_start(w0[:].rearrange("p k o -> p (k o)"), w_d[0])
            wts.append(w0)

            xT = xtpool.tile([128, S, M_C], bf16)
            w1 = wtpool.tile([128, S, O5], bf16, tag="wt")

            def xq(q):
                nc.gpsimd.dma_start(
                    xT[:, 4 * q : 4 * q + 4, :],
                    xt_d[q * 512 : (q + 1) * 512, :].rearrange(
                        "(kk p) m -> p kk m", p=128
                    ),
                )

            xg = lambda q: xT[0:1, 4 * q, 0:1]  # 1-elem probe inside quad q
            xq(0)
            guard(w0[0:1, 0, 0:1])
            xq(1)
            guard(xg(0))
            xq(2)
            guard(xg(1))
            xq(3)
            guard(xg(2))
            xq(4)
            guard(xg(3))
            xq(5)
            guard(xg(4))
            nc.gpsimd.dma_start(
                w1[:, 0:16, :].rearrange("p k o -> p (k o)"), w_d[1][:, 0 : 16 * O5]
            )
            guard(xg(5))
            xq(6)
            guard(w1[0:1, 0, 0:1])
            xq(7)
            guard(xg(6))
            nc.gpsimd.dma_start(
                w1[:, 16:32, :].rearrange("p k o -> p (k o)"), w_d[1][:, 16 * O5 :]
            )
            wts.append(w1)

            # ---------------- bias replicated to 128 partitions ----------------
            ones_bf = cpool.tile([1, 128], bf16)
            nc.vector.tensor_copy(ones_bf[:], ones_f32[:])
            b_rep = cpool.tile([128, O_C], bf16)
            for c4 in range(NOC):
                bp = pspool.tile([128, O5], f32, tag="ps")
                nc.tensor.matmul(
                    bp[:],
                    ones_bf[:],
                    b_sb[:, c4 * O5 : (c4 + 1) * O5],
                    start=True,
                    stop=True,
                )
                nc.vector.tensor_copy(b_rep[:, c4 * O5 : (c4 + 1) * O5], bp[:])

            # ---------------- adapter: wt[p, ki, j*64+e] += s*U*V ----------------
            u3 = ucols[:].rearrange("p (k j) -> p k j", j=NBO)

            def adapter(c, wt, q):  # ki chunk [8q, 8q+8)
                ad = apool.tile([128, 8, O5], bf16, tag="ad")
                ub = (
                    u3[:, q * 8 : (q + 1) * 8, c * 8 : (c + 1) * 8]
                    .rearrange("p k (j one) -> p k j one", one=1)
                    .broadcast_to([128, 8, 8, BLOCK])
                )
                vb = (
                    v_bf[:, c * O5 : (c + 1) * O5]
                    .rearrange("p (one j e) -> p one j e", one=1, j=8)
                    .broadcast_to([128, 8, 8, BLOCK])
                )
                nc.vector.tensor_mul(
                    ad[:].rearrange("p k (j e) -> p k j e", e=BLOCK), ub, vb
                )
                nc.vector.tensor_add(
                    wt[:, q * 8 : (q + 1) * 8, :], ad[:], wt[:, q * 8 : (q + 1) * 8, :]
                )

            for q in range(4):
                adapter(0, w0, q)

            # ---------------- drain helper (fused bias add) ----------------
            def drain(ps, mt, c4):
                osb = opool.tile([128, O5], f32, tag="o")
                nc.vector.scalar_tensor_tensor(
                    osb[:],
                    ps[:],
                    1.0,
                    b_rep[:, c4 * O5 : (c4 + 1) * O5],
                    op0=mult,
                    op1=addop,
                )
                nc.sync.dma_start(
                    y_d[mt * 128 : (mt + 1) * 128, c4 * O5 : (c4 + 1) * O5], osb[:]
                )

            # ---------------- phase 1: oc4=0, ki-outer (chases x quads) --------
            ps1 = [pspool.tile([128, O5], f32, tag="ps", name=f"ps1_{i}") for i in range(MT)]
            for ki in range(S):
                for mt in range(MT):
                    nc.tensor.matmul(
                        ps1[mt][:],
                        xT[:, ki, mt * 128 : (mt + 1) * 128],
                        w0[:, ki, :],
                        start=(ki == 0),
                        stop=(ki == S - 1),
                    )
            # adapter for W1 runs on DVE while phase-1 GEMM streams
            for q in range(4):
                adapter(1, w1, q)
            for mt in range(MT):
                drain(ps1[mt], mt, 0)

            # ---------------- phases 2-4: mt-outer, W double-buffered ----------
            for c4 in range(1, NOC):
                for mt in range(MT):
                    ps = pspool.tile([128, O5], f32, tag="ps")
                    for ki in range(S):
                        nc.tensor.matmul(
                            ps[:],
                            xT[:, ki, mt * 128 : (mt + 1) * 128],
                            wts[c4][:, ki, :],
                            start=(ki == 0),
                            stop=(ki == S - 1),
                        )
                    drain(ps, mt, c4)
                    if mt == 1 and c4 + 1 < NOC:
                        wn = wtpool.tile([128, S, O5], bf16, tag="wt")
                        nc.gpsimd.dma_start(
                            wn[:].rearrange("p k o -> p (k o)"), w_d[c4 + 1]
                        )
                        for q in range(4):
                            adapter(c4 + 1, wn, q)
                        wts.append(wn)

    nc.compile()
    return nc


def _get_program():
    key = "full"
    if key not in _CACHE:
        _CACHE[key] = build_program()
    return _CACHE[key]


def _prep_in_maps(x, W_base, b_base, U, V, S):
    """Host-side sharding + pure relayout (no arithmetic on values)."""
    B, N, DIN = x.shape
    xf = np.ascontiguousarray(x.reshape(B * N, DIN))

    ns = K_TOT // 128  # stripe count (module-level S is shadowed by scale input)
    # per-oc tensors (shared by the 4 data-parallel cores in each column)
    w_oc, b_oc, u_oc, v_oc = [], [], [], []
    for oc in range(GRID_O):
        WT = W_base[oc * O_C : (oc + 1) * O_C, :].T  # [K, O_C] view
        wc = np.ascontiguousarray(
            WT.reshape(ns, 128, NOC, O5).transpose(2, 1, 0, 3)
        ).reshape(NOC, 128, ns * O5)
        w_oc.append(wc)
        b_oc.append(np.ascontiguousarray(b_base[oc * O_C : (oc + 1) * O_C]))
        Uj = U[:, :, oc * NBO : (oc + 1) * NBO]  # [64 d, 64 i, NBO j]
        uc = np.ascontiguousarray(
            Uj.transpose(1, 0, 2).reshape(ns, 2, BLOCK, NBO).transpose(1, 2, 0, 3)
        ).reshape(128, ns * NBO)
        u_oc.append(uc)
        Vj = V[oc * NBO : (oc + 1) * NBO]  # [NBO j, 64 d, 64 e]
        vt = Vj.transpose(1, 0, 2).reshape(BLOCK, O_C)  # [d, j*64+e]
        v_oc.append(np.ascontiguousarray(np.concatenate([vt, vt], axis=0)))

    xt_mc = [
        np.ascontiguousarray(xf[mc * M_C : (mc + 1) * M_C, :].T)
        for mc in range(GRID_M)
    ]

    in_maps = []
    for c in range(8):
        mc, oc = divmod(c, GRID_O)
        in_maps.append(
            {
                "xt": xt_mc[mc],
                "wc": w_oc[oc],
                "bb": b_oc[oc],
                "uc": u_oc[oc],
                "vr": v_oc[oc],
                "ss": np.ascontiguousarray(S),
            }
        )
    return in_maps


def kernel(x, W_base, b_base, U, V, S):
    from concourse import bass_utils

    x = np.asarray(x, dtype=np.float32)
    W_base = np.asarray(W_base, dtype=np.float32)
    b_base = np.asarray(b_base, dtype=np.float32)
    U = np.asarray(U, dtype=np.float32)
    V = np.asarray(V, dtype=np.float32)
    S = np.asarray(S, dtype=np.float32)

    B, N, DIN = x.shape
    nc = _get_program()
    in_maps = _prep_in_maps(x, W_base, b_base, U, V, S)
    res = bass_utils.run_bass_kernel_spmd(nc, in_maps, core_ids=list(range(8)))

    y = np.empty((M_TOT, O_TOT), dtype=np.float32)
    for c in range(8):
        mc, oc = divmod(c, GRID_O)
        y[mc * M_C : (mc + 1) * M_C, oc * O_C : (oc + 1) * O_C] = res.results[c]["yy"]
    return y.reshape(B, N, O_TOT)
